# revision 1
# baseline (speedup 1.0000x reference)
"""Trainium2 Bass kernel for nn_DecoderLayer (RNMT+ LN-LSTM decoder layer).

Sharding: data-parallel over batch across the 8 NeuronCores (B=32 -> 4 rows
per core). The recurrence is sequential in time; each core runs its own batch
slice with zero cross-core communication:
  pre-phase : Z = [x, attn, 1] @ [Wx; b]  (fp32r matmuls, streamed from HBM)
  loop (256): g = Z_t + h @ Wh ; joint LayerNorm over the 4*1024 gate slab
              (per-partition bn_stats + PE indicator-matmul combine);
              sigmoid/tanh gates; c/h update; PE transposes of h for the next
              step's stationary operand.
Host only concatenates/transposes/pads inputs and reassembles the output.
"""
import sys

sys.path.insert(0, "/opt/trn_rl_repo")

import numpy as np

import concourse.bass as bass
import concourse.tile as tile
from concourse import bacc, mybir
from concourse.masks import make_identity

B, S, ISIZE, OSIZE = 32, 256, 1024, 1024
NCORES = 8
BL = B // NCORES  # 4 batch rows per core
INSZ = ISIZE + OSIZE  # x-part of W rows (2048)
KF = INSZ // 128 + 1  # 16 k-tiles + 1 bias/ones tile = 17
NG = 4 * OSIZE  # 4096 gate columns
EPS = 1e-5
F32, F32R = mybir.dt.float32, mybir.dt.float32r

_cache = {}


def build_nc(s_steps=S):
    nc = bacc.Bacc(None)
    xt_in = nc.dram_tensor("xt_in", [128, KF, BL * s_steps], F32R, kind="ExternalInput")
    wx_in = nc.dram_tensor("wx_in", [16, 128, KF, 256], F32R, kind="ExternalInput")
    wh_in = nc.dram_tensor("wh_in", [128, 8, NG], F32R, kind="ExternalInput")
    lng_in = nc.dram_tensor("lng_in", [128, OSIZE], F32, kind="ExternalInput")
    lnb_in = nc.dram_tensor("lnb_in", [128, OSIZE], F32, kind="ExternalInput")
    ind_in = nc.dram_tensor("ind_in", [128, 128], F32, kind="ExternalInput")
    res_in = nc.dram_tensor("res_in", [BL, s_steps, OSIZE], F32, kind="ExternalInput")
    ihx = nc.dram_tensor("ihx", [128, 8, BL], F32R, kind="ExternalInput")
    icx = nc.dram_tensor("icx", [BL, OSIZE], F32, kind="ExternalInput")
    out = nc.dram_tensor("out", [BL, s_steps, OSIZE], F32, kind="ExternalOutput")
    zd = nc.dram_tensor("zd", [BL * s_steps, NG], F32, kind="Internal")

    MT = BL * s_steps // 128  # bt m-tiles (8 at S=256)

    with tile.TileContext(nc) as tc:
        # ---------------- pre-phase: Z = X~ @ Wx~ ----------------
        with (
            tc.tile_pool(name="xtp", bufs=1) as xtp,
            tc.tile_pool(name="wxp", bufs=2) as wxp,
            tc.tile_pool(name="zst", bufs=3) as zst,
            tc.tile_pool(name="pps", bufs=2, space="PSUM") as pps,
        ):
            xts = xtp.tile([128, KF, BL * s_steps], F32R)
            nc.sync.dma_start(out=xts, in_=xt_in[:, :, :])
            for nb in range(16):
                wxb = wxp.tile([128, KF, 256], F32R, tag="wxb")
                nc.sync.dma_start(out=wxb, in_=wx_in[nb, :, :, :])
                for m in range(MT):
                    ps = pps.tile([128, 256], F32, tag="zps")
                    for kf in range(KF):
                        nc.tensor.matmul(
                            ps,
                            xts[:, kf, m * 128 : (m + 1) * 128],
                            wxb[:, kf, :],
                            start=(kf == 0),
                            stop=(kf == KF - 1),
                        )
                    zs = zst.tile([128, 256], F32, tag="zs")
                    nc.vector.tensor_copy(zs, ps)
                    nc.sync.dma_start(
                        out=zd[m * 128 : (m + 1) * 128, nb * 256 : (nb + 1) * 256],
                        in_=zs,
                    )

        # ---------------- recurrent loop ----------------
        with (
            tc.tile_pool(name="whp", bufs=1) as whp,
            tc.tile_pool(name="cst", bufs=1) as cst,
            tc.tile_pool(name="z16p", bufs=2) as z16p,
            tc.tile_pool(name="resp", bufs=1) as resp,
            tc.tile_pool(name="yp", bufs=1) as yp,
            tc.tile_pool(name="sp", bufs=2) as sp,
            tc.tile_pool(name="hp", bufs=1) as hp,
            tc.tile_pool(name="op", bufs=1) as op,
            tc.tile_pool(name="htp", bufs=2) as htp,
            tc.tile_pool(name="gps", bufs=2, space="PSUM") as gps,
            tc.tile_pool(name="tps", bufs=1, space="PSUM") as tps,
            tc.tile_pool(name="sps", bufs=1, space="PSUM") as sps,
        ):
            whs = whp.tile([128, 8, NG], F32R)
            nc.sync.dma_start(out=whs, in_=wh_in[:, :, :])
            lng = whp.tile([128, OSIZE], F32)
            nc.sync.dma_start(out=lng, in_=lng_in[:, :])
            lnb = whp.tile([128, OSIZE], F32)
            nc.sync.dma_start(out=lnb, in_=lnb_in[:, :])
            ind = whp.tile([128, 128], F32)
            nc.sync.dma_start(out=ind, in_=ind_in[:, :])
            epst = whp.tile([128, 1], F32)
            nc.vector.memset(epst, EPS)
            gsb = whp.tile([128, OSIZE], F32)
            nc.vector.memset(gsb, 0.0)
            id4 = whp.tile([4, 4], F32)
            make_identity(nc, id4)

            # c state [4, 1024]; initial h transposed [128, 8, 4]
            c = cst.tile([BL, OSIZE], F32)
            nc.sync.dma_start(out=c, in_=icx[:, :])
            hT = htp.tile([128, 8, BL], F32R, tag="hT")
            nc.sync.dma_start(out=hT, in_=ihx[:, :, :])

            zview = zd.ap().rearrange("(b t) (g p) -> t g b p", b=BL, g=4)

            for t in range(s_steps):
                z16 = z16p.tile([128, OSIZE], F32, tag="z16")
                for g4 in range(4):
                    nc.sync.dma_start(
                        out=z16[32 * g4 : 32 * g4 + BL, :], in_=zview[t, g4]
                    )
                rest = resp.tile([BL, OSIZE], F32, tag="rest")
                nc.sync.dma_start(out=rest, in_=res_in[:, t, :])

                for c4 in range(4):
                    ps = gps.tile([BL, OSIZE], F32, tag="gps")
                    for kk in range(8):
                        for n2 in range(2):
                            nc.tensor.matmul(
                                ps[:, n2 * 512 : (n2 + 1) * 512],
                                hT[:, kk, :],
                                whs[
                                    :, kk,
                                    c4 * 1024 + n2 * 512 : c4 * 1024 + (n2 + 1) * 512,
                                ],
                                start=(kk == 0),
                                stop=(kk == 7),
                            )
                    nc.vector.tensor_add(
                        gsb[32 * c4 : 32 * c4 + BL, :],
                        ps,
                        z16[32 * c4 : 32 * c4 + BL, :],
                    )

                # LayerNorm stats: per-partition bn over 1024, PE combine over
                # the 4 gate partitions of each batch row
                stats = sp.tile([128, 2, nc.vector.BN_STATS_DIM], F32, tag="stats")
                gre = gsb.rearrange("p (s f) -> p s f", s=2)
                for s2 in range(2):
                    nc.vector.bn_stats(out=stats[:, s2, :], in_=gre[:, s2, :])
                mv = sp.tile([128, 2], F32, tag="mv")
                nc.vector.bn_aggr(out=mv, in_=stats)
                msq = sp.tile([128, 1], F32, tag="msq")
                nc.vector.tensor_mul(msq, mv[:, 0:1], mv[:, 0:1])
                nc.vector.tensor_add(mv[:, 1:2], mv[:, 1:2], msq)
                pss = sps.tile([128, 2], F32, tag="pss")
                nc.tensor.matmul(
                    pss, ind, mv, start=True, stop=True
                )
                mu = sp.tile([128, 1], F32, tag="mu")
                nc.scalar.mul(out=mu, in_=pss[:, 0:1], mul=0.25)
                e2 = sp.tile([128, 1], F32, tag="e2")
                nc.scalar.mul(out=e2, in_=pss[:, 1:2], mul=0.25)
                nc.vector.tensor_mul(msq, mu, mu)
                var = sp.tile([128, 1], F32, tag="var")
                nc.vector.tensor_sub(var, e2, msq)
                sd = sp.tile([128, 1], F32, tag="sd")
                nc.scalar.activation(
                    out=sd, in_=var, func=mybir.ActivationFunctionType.Sqrt,
                    bias=epst, scale=1.0,
                )
                rstd = sp.tile([128, 1], F32, tag="rstd")
                nc.vector.reciprocal(out=rstd, in_=sd)

                y = yp.tile([128, OSIZE], F32, tag="y")
                nc.vector.tensor_scalar(
                    out=y, in0=gsb, scalar1=mu, scalar2=rstd,
                    op0=mybir.AluOpType.subtract, op1=mybir.AluOpType.mult,
                )
                nc.vector.tensor_mul(y, y, lng)
                nc.vector.tensor_add(y, y, lnb)
                y2 = yp.tile([BL, 4, OSIZE], F32, tag="y2")
                for g4 in range(3):
                    nc.scalar.activation(
                        out=y2[:, g4, :], in_=y[32 * g4 : 32 * g4 + BL, :],
                        func=mybir.ActivationFunctionType.Sigmoid,
                    )
                nc.scalar.activation(
                    out=y2[:, 3, :], in_=y[96 : 96 + BL, :],
                    func=mybir.ActivationFunctionType.Tanh,
                )

                t1 = cst.tile([BL, OSIZE], F32, tag="t1")
                nc.vector.tensor_mul(t1, y2[:, 0, :], y2[:, 3, :])
                nc.vector.tensor_mul(c, y2[:, 1, :], c)
                nc.vector.tensor_add(c, c, t1)
                h = hp.tile([BL, OSIZE], F32, tag="h")
                nc.vector.tensor_mul(h, y2[:, 2, :], c)

                osb = op.tile([BL, OSIZE], F32, tag="osb")
                nc.vector.tensor_add(osb, h, rest)
                nc.sync.dma_start(out=out[:, t, :], in_=osb)

                if t + 1 < s_steps:
                    psT = tps.tile([128, 8, BL], F32, tag="psT")
                    for kk in range(8):
                        nc.tensor.transpose(
                            psT[:, kk, :], h[:, kk * 128 : (kk + 1) * 128], id4
                        )
                    hT = htp.tile([128, 8, BL], F32R, tag="hT")
                    nc.vector.tensor_copy(hT, psT)
    nc.finalize()
    return nc


def _prep(inputo, attn, W, b, ln_g, ln_b, init_hx, init_cx, s_steps=S):
    inputo = np.asarray(inputo, np.float32)
    attn = np.asarray(attn, np.float32)
    W = np.asarray(W, np.float32)
    b = np.asarray(b, np.float32)
    ln_g = np.asarray(ln_g, np.float32)
    ln_b = np.asarray(ln_b, np.float32)
    X = np.concatenate([inputo, attn], axis=-1)  # [B, S, 2048]

    # Wx~ [17, 128, 4096]: rows 0..2047 of W, then the bias row + zero pad
    wx_ext = np.zeros((KF * 128, NG), np.float32)
    wx_ext[:INSZ] = W[:INSZ]
    wx_ext[INSZ] = b
    wx_blocks = np.ascontiguousarray(
        wx_ext.reshape(KF, 128, 16, 256).transpose(2, 1, 0, 3)
    )  # [16, 128, 17, 256]
    wh_perm = np.ascontiguousarray(
        W[INSZ:].reshape(8, 128, NG).transpose(1, 0, 2)
    )  # [128, 8, 4096]
    lng16 = np.ascontiguousarray(np.repeat(ln_g, 32, axis=0))  # [128, 1024]
    lnb16 = np.ascontiguousarray(np.repeat(ln_b, 32, axis=0))
    pp = np.arange(128)
    ind16 = ((pp[:, None] % 32 == pp[None, :] % 32) & (pp[:, None] % 32 < 4)).astype(
        np.float32
    )
    ihx0 = np.asarray(init_hx, np.float32).reshape(OSIZE)
    ihx = np.ascontiguousarray(
        np.broadcast_to(ihx0.reshape(8, 128).T[:, :, None], (128, 8, BL))
    )
    icx = np.ascontiguousarray(
        np.broadcast_to(np.asarray(init_cx, np.float32).reshape(1, OSIZE), (BL, OSIZE))
    )

    in_maps = []
    for k in range(NCORES):
        Xk = X[k * BL : (k + 1) * BL, :s_steps]  # [4, s, 2048]
        xt = np.zeros((KF * 128, BL * s_steps), np.float32)
        xt[:INSZ] = Xk.reshape(BL * s_steps, INSZ).T
        xt[INSZ] = 1.0
        xtk = np.ascontiguousarray(
            xt.reshape(KF, 128, BL * s_steps).transpose(1, 0, 2)
        )  # [128, 17, 4*s]
        in_maps.append(
            dict(
                xt_in=xtk,
                wx_in=wx_blocks,
                wh_in=wh_perm,
                lng_in=lng16,
                lnb_in=lnb16,
                ind_in=ind16,
                res_in=np.ascontiguousarray(inputo[k * BL : (k + 1) * BL, :s_steps]),
                ihx=ihx,
                icx=icx,
            )
        )
    return in_maps


def kernel(inputo, attn, W, b, ln_g, ln_b, init_hx, init_cx):
    from concourse.bass_utils import run_bass_kernel_spmd

    if "nc" not in _cache:
        _cache["nc"] = build_nc(S)
    in_maps = _prep(inputo, attn, W, b, ln_g, ln_b, init_hx, init_cx, S)
    res = run_bass_kernel_spmd(
        _cache["nc"], in_maps, core_ids=list(range(NCORES))
    )
    return np.concatenate([r["out"] for r in res.results], axis=0)

